# revision 38
# baseline (speedup 1.0000x reference)
"""Trainium2 Bass kernel for nn_CrossAttention (B=8, N1=64, N2=4096, C=768, H=12).

Strategy: data-parallel over batch across 8 NeuronCores (one item per core,
no collectives). All activations kept transposed (channels on partitions,
tokens on the free dim) so every matmul contracts over SBUF partitions.

Key algebraic restructurings (exploiting that the module's combine with v is
ELEMENTWISE, attn_t * v, not attn @ v):

  1. scores_h = q_h @ k_h^T = (q_h @ W_k_h) @ yT = A_h @ yT.  A = qT^T @ W_k
     is a tiny [768,768] precompute; scores then contract over the full
     K=128 partitions with the SAME moving operand (yT chunks) as the
     v-projection - k is never materialized.
  2. softmax normalization is deferred: U_h = exp(s_h) * v_h is accumulated
     unnormalized; row-sums come free via ACT's fused accum_out; 1/S is
     folded into the projection weights (O(C^2), not O(C*N2)).
  3. U8 = e4m3(8*E*v) hi/lo splits are built on the fly (DVE multiply,
     Pool/ACT cast, DVE residual) so the out-projection also runs fp8.

fp8 acceleration (DoubleRow perf mode, 2 contraction k-tiles per pass):
  - v-projection and output projection run as error-compensated fp8
    "double-double": hi/lo e4m3 splits of both operands, accumulating
    hi.hi + lo.hi + hi.lo (9 DoubleRow passes vs 12 bf16-equivalent).
    Quantization error ~eps^2.
  - scores run in plain fp8: A is quantized on device (x128) against the
    hi yT split; score errors are absolute-small and further damped by
    softmax's exp. End-to-end rel err ~1.3e-2 (gate 2e-2).

The token dim streams in 1024-wide chunks with scores/vproj units
interleaved; the last chunk's scores (exps + 1/S fold + wps8 quantization)
are hoisted BEFORE its v-projection so the out-projection starts with no
boundary stall, and that chunk's U quantization runs inside the
out-projection phase.
"""

import numpy as np
import ml_dtypes

import concourse.bass as bass
import concourse.mybir as mybir
import concourse.tile as tile
from concourse import bacc
from concourse.bass_utils import run_bass_kernel_spmd

BF16 = mybir.dt.bfloat16
FP8 = mybir.dt.float8e4
F32 = mybir.dt.float32
DR = mybir.MatmulPerfMode.DoubleRow

B, N1, N2, C, H = 8, 64, 4096, 768, 12
HD = C // H              # 64
SCALE = HD ** -0.5       # 1/8
CT = C // 128            # 6 partition tiles of channels
CT2 = 2 * CT             # 12 fp8 k-tiles (hi | lo)
CW = 1024                # streamed token-chunk width
NCH = N2 // CW           # 4 chunks
PAIRS = CT               # 6 head pairs (2 heads per 128-partition tile)

# quantization scales (powers of two; folded back out exactly)
SY = 16.0                # yT shipped as e4m3(16*y) + residual
SWV = 64.0               # W_v^T shipped as e4m3(64*Wv) + residual
SA = 128.0               # A quantized on device as e4m3(128*A)
ALPHA = 8.0              # U = (8E) * v: fp8 range for U
EXPB = float(np.log(ALPHA))      # exp bias ln(8)
VDRAIN = 1.0 / (SY * SWV)        # 2^-10: v psum -> true v
# qbd folds 1/8, A carries x128, y_hi carries x16 -> psum_s = 2048*s
EXPSCALE = 1.0 / (SA * SY)       # 2^-11
# S_parts accumulate 8*S, R = 1/(8S); shipping W_proj^T * 2^18 puts
# wps8 = wp * 2^18/(8S) (rms ~0.15) and the out psum at 2^18 * out_pre.
SWP = 2.0 ** 18                  # W_proj^T pre-scale
SD = 2.0 ** -18                  # out-proj psum drain scale

BUFS_YT = 2
BUFS_E = 6

_CACHE = {}


def _build():
    nc = bacc.Bacc("TRN2", target_bir_lowering=False, debug=False)

    xT_d = nc.dram_tensor("xT", [C, N1], BF16, kind="ExternalInput")
    # yq: rows 0..C-1 = e4m3(16*yT), rows C..2C-1 = residual
    yq_d = nc.dram_tensor("yq", [2 * C, N2], FP8, kind="ExternalInput")
    # wq: W_q^T layout [c_in, c_out]
    wq_d = nc.dram_tensor("wq", [C, C], BF16, kind="ExternalInput")
    # wk: natural layout [c_out, c_in] = W_qkv[C:2C, :]
    wk_d = nc.dram_tensor("wk", [C, C], BF16, kind="ExternalInput")
    # wvq: [Wv_hi | Wv_lo] stacked on rows; each [c_in, c_out] = e4m3(64*Wv^T)
    wvq_d = nc.dram_tensor("wvq", [2 * C, C], FP8, kind="ExternalInput")
    wprojT_d = nc.dram_tensor("wprojT", [C, C], BF16, kind="ExternalInput")
    bproj_d = nc.dram_tensor("bproj", [C, 1], F32, kind="ExternalInput")
    outT_d = nc.dram_tensor("outT", [C, N2], BF16, kind="ExternalOutput")

    def t6(ap):  # [768, X] dram view -> [128, 6, X] partition-tiled view
        return ap.rearrange("(t p) c -> p t c", p=128)

    with tile.TileContext(nc) as tc:
        with (
            tc.tile_pool(name="persist", bufs=1) as pp,
            tc.tile_pool(name="work", bufs=2) as wp,
            tc.tile_pool(name="psum", bufs=2, space=bass.MemorySpace.PSUM) as psp,
        ):
            # ---- persistent tiles (partition-tiled: [:, kk, :] = rows of 128)
            wq_sb = pp.tile([128, CT, C], BF16, name="wq", tag="wq")
            wv_sb = pp.tile([128, CT2, C], FP8, name="wv", tag="wv")
            wk_sb = pp.tile([128, CT, C], BF16, name="wk", tag="wk")
            wp_sb = pp.tile([128, CT, C], BF16, name="wpr", tag="wpr")
            wps_sb = pp.tile([128, CT, C], BF16, name="wps", tag="wps")
            A_sb = pp.tile([128, CT, C], FP8, name="A", tag="A")
            xT_sb = pp.tile([128, CT, N1], BF16, name="xTs", tag="xTs")
            bias_sb = pp.tile([128, CT, 1], F32, name="biass", tag="biass")
            # block-diagonal q: qbd[0:64, g, 0:64] = qT head 2g,
            # qbd[64:128, g, 64:128] = qT head 2g+1, zeros elsewhere.
            qbd = pp.tile([128, CT, 128], BF16, name="qbd", tag="qbd")
            U8 = pp.tile([128, CT2, N2], FP8, name="U8", tag="U8")
            wps8 = pp.tile([128, CT2, C], FP8, name="wps8", tag="wps8")
            S_parts = [pp.tile([128, NCH], F32, name=f"Sp{g}", tag=f"Sp{g}")
                       for g in range(PAIRS)]
            lbias = pp.tile([128, 1], F32, name="lbias", tag="lbias")
            nc.gpsimd.memset(lbias[:], EXPB)
            nc.gpsimd.memset(qbd[:], 0.0)

            # ---- batched weight/input DMAs ----------------------------------
            # ALL input transfers ride the sync (SP) queue: transfers
            # serialize on the shared DMA engines, so a single queue pins the
            # global order to the PE program's consumption order. Output
            # stores use other queues (disjoint in time).
            nc.sync.dma_start(wv_sb[:, :CT, :], t6(wvq_d[:C, :]))

            def chunk_dma(ci):
                yq_c = wp.tile([128, CT2, CW], FP8, name="yqc", tag="yqc",
                               bufs=BUFS_YT)
                cols = slice(CW * ci, CW * (ci + 1))
                nc.sync.dma_start(
                    yq_c[:],
                    yq_d[:, cols].rearrange("(t p) c -> p t c", p=128))
                return yq_c

            # chunk 0 in hi / wv_lo / lo arrival order = vproj pass order.
            yq_next = wp.tile([128, CT2, CW], FP8, name="yqc", tag="yqc",
                              bufs=BUFS_YT)
            nc.sync.dma_start(yq_next[:, :CT, :], t6(yq_d[:C, :CW]))
            nc.sync.dma_start(wv_sb[:, CT:, :], t6(wvq_d[C:, :]))
            nc.sync.dma_start(yq_next[:, CT:, :], t6(yq_d[C:, :CW]))

            nc.sync.dma_start(xT_sb[:], t6(xT_d[:, :]))
            nc.sync.dma_start(wq_sb[:], t6(wq_d[:, :]))
            nc.sync.dma_start(wk_sb[:], t6(wk_d[:, :]))

            def vproj_m(pskv, yq_c, m, terms):
                npass = 0
                total = 3 * len(terms)
                for (lb, rb) in terms:
                    for j in range(3):
                        for hf in range(2):
                            nc.tensor.matmul(
                                pskv[:, 512 * hf:512 * (hf + 1)],
                                wv_sb[:, lb + 2 * j:lb + 2 * j + 2,
                                      128 * m:128 * (m + 1)],
                                yq_c[:, rb + 2 * j:rb + 2 * j + 2,
                                     512 * hf:512 * (hf + 1)],
                                start=(npass == 0), stop=(npass == total - 1),
                                perf_mode=DR,
                            )
                        npass += 1

            def vdrain(vT_c, pskv, m):
                if m % 2 == 0:
                    nc.scalar.activation(vT_c[m][:], pskv[:],
                                         mybir.ActivationFunctionType.Copy,
                                         bias=0.0, scale=VDRAIN)
                else:
                    nc.vector.tensor_scalar_mul(vT_c[m][:], pskv[:], VDRAIN)

            def vproj_units(yq_c):
                vT_c = [wp.tile([128, CW], BF16, name=f"vTc{m}",
                                tag=f"vTc{m}", bufs=2) for m in range(CT)]

                def unit(m):
                    pskv = psp.tile([128, CW], F32, name="pskv", tag="pskv",
                                    bufs=2)
                    vproj_m(pskv, yq_c, m, [(0, 0), (CT, 0), (0, CT)])
                    vdrain(vT_c, pskv, m)
                return vT_c, [lambda m=m: unit(m) for m in range(CT)]

            def vproj(yq_c):
                vT_c, units = vproj_units(yq_c)
                for u in units:
                    u()
                return vT_c

            def vproj0(yq_c):
                """Chunk-0 variant ordered around DMA arrival: all hi terms
                for m0-m2 run before wv_lo/y_lo land; the lo passes close
                after. Holds 3 psums (the full pskv ring)."""
                vT_c = [wp.tile([128, CW], BF16, name=f"vTc{m}",
                                tag=f"vTc{m}", bufs=2) for m in range(CT)]
                ps = [psp.tile([128, CW], F32, name="pskv", tag="pskv",
                               bufs=2) for _ in range(2)]
                for (lb, rb) in [(0, 0), (CT, 0)]:
                    for m in range(2):
                        for j in range(3):
                            for hf in range(2):
                                nc.tensor.matmul(
                                    ps[m][:, 512 * hf:512 * (hf + 1)],
                                    wv_sb[:, lb + 2 * j:lb + 2 * j + 2,
                                          128 * m:128 * (m + 1)],
                                    yq_c[:, rb + 2 * j:rb + 2 * j + 2,
                                         512 * hf:512 * (hf + 1)],
                                    start=(lb == 0 and j == 0), stop=False,
                                    perf_mode=DR,
                                )
                for m in range(2):
                    for j in range(3):
                        for hf in range(2):
                            nc.tensor.matmul(
                                ps[m][:, 512 * hf:512 * (hf + 1)],
                                wv_sb[:, 2 * j:2 * j + 2,
                                      128 * m:128 * (m + 1)],
                                yq_c[:, CT + 2 * j:CT + 2 * j + 2,
                                     512 * hf:512 * (hf + 1)],
                                start=False, stop=(j == 2),
                                perf_mode=DR,
                            )
                    vdrain(vT_c, ps[m], m)
                for m in range(2, CT):
                    pskv = psp.tile([128, CW], F32, name="pskv", tag="pskv",
                                    bufs=2)
                    vproj_m(pskv, yq_c, m, [(0, 0), (CT, 0), (0, CT)])
                    vdrain(vT_c, pskv, m)
                return vT_c

            def fold_wps(g):
                # fold 1/S into the projection weights for head pair g and
                # quantize them hi/lo to fp8
                S_tot = wp.tile([128, 1], F32, name="S_tot", tag="S_tot",
                                bufs=2)
                nc.vector.tensor_reduce(S_tot[:], S_parts[g][:],
                                        axis=mybir.AxisListType.X,
                                        op=mybir.AluOpType.add)
                R_g = wp.tile([128, 1], F32, name="R_g", tag="R_g", bufs=2)
                nc.vector.reciprocal(R_g[:], S_tot[:])
                nc.vector.tensor_scalar_mul(wps_sb[:, g, :], wp_sb[:, g, :],
                                            R_g[:])
                nc.gpsimd.tensor_copy(wps8[:, g, :], wps_sb[:, g, :])
                nc.vector.tensor_tensor(wps8[:, CT + g, :], wps_sb[:, g, :],
                                        wps8[:, g, :],
                                        op=mybir.AluOpType.subtract)

            def scores_unit(ci, yq_c, vT_c, g, deferred, defer):
                if True:
                    pss = psp.tile([128, CW], F32, name="pss", tag="pss",
                                   bufs=2)
                    for j in range(3):
                        for hf in range(2):
                            nc.tensor.matmul(
                                pss[:, 512 * hf:512 * (hf + 1)],
                                A_sb[:, 2 * j:2 * j + 2, 128 * g:128 * (g + 1)],
                                yq_c[:, 2 * j:2 * j + 2,
                                     512 * hf:512 * (hf + 1)],
                                start=(j == 0), stop=(j == 2),
                                perf_mode=DR,
                            )
                    e_sb = wp.tile([128, CW], BF16, name="e_sb", tag="e_sb",
                                   bufs=BUFS_E)
                    nc.scalar.activation(e_sb[:], pss[:],
                                         mybir.ActivationFunctionType.Exp,
                                         bias=lbias[:], scale=EXPSCALE,
                                         accum_out=S_parts[g][:, ci:ci + 1])
                    tok = slice(CW * ci, CW * (ci + 1))

                    def uquant(vT_, g=g, e_sb=e_sb, tok=tok):
                        P_t = wp.tile([128, CW], BF16, name="pmul",
                                      tag="pmul", bufs=3)
                        nc.vector.tensor_mul(P_t[:], e_sb[:], vT_[g][:])
                        if g % 2 == 0:
                            nc.gpsimd.tensor_copy(U8[:, g, tok], P_t[:])
                        else:
                            nc.scalar.activation(
                                U8[:, g, tok], P_t[:],
                                mybir.ActivationFunctionType.Copy,
                                bias=0.0, scale=1.0)
                        nc.vector.tensor_tensor(U8[:, CT + g, tok], P_t[:],
                                                U8[:, g, tok],
                                                op=mybir.AluOpType.subtract)

                    if defer:
                        # row-sums for g are complete: fold 1/S + quantize
                        # the projection weights now; U waits for the v-psums
                        fold_wps(g)
                        deferred.append(uquant)
                    else:
                        uquant(vT_c)

            def scores(ci, yq_c, vT_c, defer=False):
                deferred = []
                for g in range(PAIRS):
                    scores_unit(ci, yq_c, vT_c, g, deferred, defer)
                return deferred

            # ---- chunk-0 v-projection first: its inputs lead the DMA stream
            vT_next = vproj0(yq_next)

            # ---- qT = (W_q @ xT) * scale ------------------------------------
            for m in range(CT):
                psq = psp.tile([128, N1], F32, name="psq", tag="pss", bufs=2)
                for kk in range(CT):
                    nc.tensor.matmul(
                        psq[:],
                        wq_sb[:, kk, 128 * m:128 * (m + 1)],
                        xT_sb[:, kk, :],
                        start=(kk == 0), stop=(kk == CT - 1),
                    )
                nc.scalar.activation(qbd[0:64, m, 0:64], psq[0:64, :],
                                     mybir.ActivationFunctionType.Copy,
                                     bias=0.0, scale=SCALE)
                nc.scalar.activation(qbd[64:128, m, 64:128], psq[64:128, :],
                                     mybir.ActivationFunctionType.Copy,
                                     bias=0.0, scale=SCALE)

            # ---- A_h = q_h @ W_k_h, quantized to e4m3(128*A) ----------------
            for kk in range(CT):
                psA = psp.tile([128, C], F32, name="psA", tag="pss", bufs=2)
                for g in range(PAIRS):
                    nc.tensor.matmul(
                        psA[:, 128 * g:128 * (g + 1)],
                        wk_sb[:, g, 128 * kk:128 * (kk + 1)],
                        qbd[:, g, :],
                        start=True, stop=True,
                    )
                if kk % 2 == 0:
                    nc.scalar.activation(A_sb[:, kk, :], psA[:],
                                         mybir.ActivationFunctionType.Copy,
                                         bias=0.0, scale=SA)
                else:
                    nc.vector.tensor_scalar_mul(A_sb[:, kk, :], psA[:], SA)

            # ---- stream over token chunks -----------------------------------
            # The last chunk's scores (exps + folds) run BEFORE its
            # v-projection so the out-projection starts without waiting on
            # the 1/S fold chain.
            def interleave(ci, yq_c, vT_c, yq_n, defer=False):
                """Emit scores(ci) g-units interleaved with vproj m-units
                of the next chunk: g0 g1 m0 g2 m1 g3 m2 g4 m3 g5 m4 m5.
                Smooths psum-ring pressure and engine queue order."""
                deferred = []
                vT_n, units = (vproj_units(yq_n) if yq_n is not None
                               else (None, []))
                for g in range(PAIRS):
                    scores_unit(ci, yq_c, vT_c, g, deferred, defer)
                    if g >= 1 and g - 1 < len(units):
                        units[g - 1]()
                for m in range(PAIRS - 1, CT):
                    if m < len(units):
                        units[m]()
                return vT_n, deferred

            for ci in range(NCH - 1):
                yq_c, vT_c = yq_next, vT_next
                yq_next = chunk_dma(ci + 1)
                if ci == 1:
                    # proj weights: mid-stream, after the tight early chunks
                    nc.sync.dma_start(wp_sb[:], t6(wprojT_d[:, :]))
                    nc.sync.dma_start(bias_sb[:], t6(bproj_d[:, :]))
                if ci + 1 < NCH - 1:
                    vT_next, _ = interleave(ci, yq_c, vT_c, yq_next)
                else:
                    interleave(ci, yq_c, vT_c, None)
            deferred = scores(NCH - 1, yq_next, None, defer=True)
            vT_last = vproj(yq_next)

            # ---- outT = (W_proj_scaled @ U) * 2^-18 + b ---------------------
            NB = N2 // 1024  # four 1024-token output blocks
            for n in range(NB):
                tok = slice(1024 * n, 1024 * (n + 1))
                last = (n == NB - 1)
                outc = None
                for m in range(CT):
                    if m % 3 == 0 and not last:
                        outc = wp.tile([128, 3, 1024], BF16, name="outc",
                                       tag="outc", bufs=3)
                    psq2 = psp.tile([128, 1024], F32, name="psq2",
                                    tag="pskv", bufs=2)
                    ps2 = [psq2[:, :512], psq2[:, 512:]]
                    for ti, (lb, rb) in enumerate([(0, 0), (0, CT), (CT, 0)]):
                        for j in range(3):
                            for hf in range(2):
                                nc.tensor.matmul(
                                    ps2[hf],
                                    wps8[:, lb + 2 * j:lb + 2 * j + 2,
                                         128 * m:128 * (m + 1)],
                                    U8[:, rb + 2 * j:rb + 2 * j + 2,
                                       1024 * n + 512 * hf:
                                       1024 * n + 512 * (hf + 1)],
                                    start=(ti == 0 and j == 0),
                                    stop=(ti == 2 and j == 2),
                                    perf_mode=DR,
                                )
                    if last:
                        outm = wp.tile([128, 1024], BF16, name="outm",
                                       tag="outm", bufs=3)
                        if m == CT - 1:
                            nc.scalar.activation(
                                outm[:, :512], psq2[:, :512],
                                mybir.ActivationFunctionType.Identity,
                                bias=bias_sb[:, m, :], scale=SD)
                            nc.vector.tensor_scalar(
                                outm[:, 512:], psq2[:, 512:], SD,
                                bias_sb[:, m, :], op0=mybir.AluOpType.mult,
                                op1=mybir.AluOpType.add)
                        elif m % 2 == 0:
                            nc.scalar.activation(
                                outm[:], psq2[:],
                                mybir.ActivationFunctionType.Identity,
                                bias=bias_sb[:, m, :], scale=SD)
                        else:
                            nc.vector.tensor_scalar(
                                outm[:], psq2[:], SD, bias_sb[:, m, :],
                                op0=mybir.AluOpType.mult,
                                op1=mybir.AluOpType.add)
                        if m == CT - 1:
                            nc.sync.dma_start(
                                outT_d[128 * m:128 * (m + 1),
                                       1024 * n:1024 * n + 512],
                                outm[:, :512])
                            nc.sync.dma_start(
                                outT_d[128 * m:128 * (m + 1),
                                       1024 * n + 512:1024 * (n + 1)],
                                outm[:, 512:])
                        else:
                            nc.scalar.dma_start(
                                outT_d[128 * m:128 * (m + 1), tok], outm[:])
                    else:
                        if m % 2 == 0:
                            nc.scalar.activation(
                                outc[:, m % 3, :], psq2[:],
                                mybir.ActivationFunctionType.Identity,
                                bias=bias_sb[:, m, :], scale=SD)
                        else:
                            nc.vector.tensor_scalar(
                                outc[:, m % 3, :], psq2[:], SD,
                                bias_sb[:, m, :], op0=mybir.AluOpType.mult,
                                op1=mybir.AluOpType.add)
                        if m % 3 == 2:
                            h3 = m // 3
                            nc.scalar.dma_start(
                                outT_d[384 * h3:384 * (h3 + 1), tok].rearrange(
                                    "(t p) c -> p t c", p=128),
                                outc[:])
                if n == 0:
                    # last-chunk U quantization runs during the outproj phase
                    for fn_ in deferred:
                        fn_(vT_last)

    nc.compile()
    return nc


def kernel(x, y, W_qkv, W_proj, b_proj):
    if "nc" not in _CACHE:
        _CACHE["nc"] = _build()
    nc = _CACHE["nc"]
    in_maps = make_in_maps(x, y, W_qkv, W_proj, b_proj)
    # The axon-tunneled devices occasionally fail one execution with a
    # transient NRT_EXEC_UNIT_UNRECOVERABLE; a clean retry succeeds.
    last_err = None
    for attempt in range(3):
        try:
            res = run_bass_kernel_spmd(nc, in_maps, core_ids=list(range(B)))
            break
        except Exception as e:  # noqa: BLE001
            last_err = e
            import time
            time.sleep(2.0 * (attempt + 1))
    else:
        raise last_err
    out = np.empty((B, N2, C), np.float32)
    for i in range(B):
        out[i] = res.results[i]["outT"].astype(np.float32).T
    return out


def make_in_maps(x, y, W_qkv, W_proj, b_proj):
    bf = ml_dtypes.bfloat16
    f8 = ml_dtypes.float8_e4m3

    def q8(a):
        hi = a.astype(f8)
        lo = (a - hi.astype(np.float32)).astype(f8)
        return hi, lo

    W_qkv = np.asarray(W_qkv, np.float32)
    wq = np.ascontiguousarray(W_qkv[:C].T).astype(bf)
    wk = np.ascontiguousarray(W_qkv[C:2 * C]).astype(bf)
    wv_hi, wv_lo = q8(np.ascontiguousarray(W_qkv[2 * C:].T) * SWV)
    wvq = np.concatenate([wv_hi, wv_lo], axis=0)
    wprojT = np.ascontiguousarray(
        np.asarray(W_proj, np.float32).T * SWP).astype(bf)
    bproj = np.asarray(b_proj, np.float32).reshape(C, 1)

    in_maps = []
    for i in range(B):
        xT = np.ascontiguousarray(np.asarray(x[i], np.float32).T).astype(bf)
        yT = np.ascontiguousarray(np.asarray(y[i], np.float32).T) * SY
        y_hi, y_lo = q8(yT)
        yq = np.concatenate([y_hi, y_lo], axis=0)
        in_maps.append({
            "xT": xT,
            "yq": yq,
            "wq": wq,
            "wk": wk,
            "wvq": wvq,
            "wprojT": wprojT,
            "bproj": bproj,
        })
    return in_maps
